# revision 15
# baseline (speedup 1.0000x reference)
"""Trainium2 Bass kernel for CommittorNetBP (pairwise min-image env sum + tiny MLP).

Mathematically equivalent reformulation of the reference:

 1. Per-component wrapped squared displacement is periodic (L=10):
    wrap(dx)^2 ~= B0 + sum_n Bn cos(2*pi*n*dx/L)  (N=16 harmonics,
    weighted LS, exact at dx=0).  Via product-to-sum, d2[i,j] becomes an
    inner product of trig embeddings E[k, j] = sin(2*pi(n x_j/L + phase_k)).
    The HOST supplies uint16-quantized pre-wrapped phases so the device
    computes E with a single Sin activation per chunk (arg in [-pi, pi],
    inside the ACT table's accurate range) - no wrap ops, no phase matmul.
 2. Envelope f(t) = exp(-t)*0.5*(1+cos(pi*sqrt(t)/RC)) (t=d2, 0 beyond RC^2)
    fitted as  w0 + w1*e^{-a t} + w2*e^{-2a t}  (max fit err ~1e-3).  Only
    e^{-a t} needs the ACT engine; the square rides in a fused DVE
    scalar_tensor_tensor:  y = (e + w1/w2)*e,  with accum_out producing the
    per-particle row sums for free.  w2 is folded into W1 on the host.
 3. Pair symmetry: only upper-triangle 128-blocks are computed (10/16 of
    the work).  Diagonal blocks are internally symmetric, so row sums cover
    them; off-diagonal column sums run on the TensorEngine as ones-vector
    matmuls accumulating into a [16, 512] PSUM tile (one row per batch).
 4. MLP tail: h = relu(it @ (w2 W1).T + b1'), out = 0.5 + 0.5 tanh(z/2).

Sharding: pure data parallel, batch 128 -> 8 cores x 16.
"""

import numpy as np

# ---------------------------------------------------------------- constants
L = 10.0
PI = float(np.pi)
NP = 512
BTOT = 128
NCORES = 8
BLOC = BTOT // NCORES  # 16
NH = 16
# big harmonics are split across phase-shifted duplicate rows so the fp32r
# (fp22-multiply) truncation errors of their large products decorrelate
SPLIT = {1: 4, 2: 2}
NROW = sum(SPLIT.get(n, 1) for n in range(1, NH + 1))  # 20 (n, copy) pairs
K = 6 * NROW           # 120 embedding rows (harmonics only; B0 in exp bias)
NUM_NODES = 256

# wrap2(dx) ~= B_HARM[0] + sum_n B_HARM[n] cos(2 pi n dx / L)  (see fit.py)
B_HARM = [
    8.333339280985602, -10.132262052817431, 2.533031987461954,
    -1.1259256629152965, 0.6332712496421974, -0.4054674193923079,
    0.28152209754388213, -0.20693554768934222, 0.1581492579818848,
    -0.12482935477918758, 0.10103959650515294, -0.08452587843994076,
    0.07259494648571503, -0.06262855418061723, 0.04970117519765284,
    -0.03159276455999903, 0.011517642970082422,
]

def _spec():
    # (harmonic n, copy phase psi, weight) — psi cancels in the
    # product-to-sum identity, so any value decorrelates quantization
    out = []
    for n in range(1, NH + 1):
        reps = SPLIT.get(n, 1)
        for c in range(reps):
            out.append((n, c * 0.2471, B_HARM[n] / reps))
    return out

# envelope fit: f(t) ~= EW0 + EW1 exp(-A t) + EW2 exp(-2 A t)
EW0 = 4.218244372734287e-05
EW1 = -0.057432602447565484
EW2 = 1.0583648509498493
A = 0.6827232177720551
CC = EW1 / EW2         # stt scalar: y = (e + CC) * e
B0C = 3.0 * B_HARM[0]  # constant part of d2a, folded into the exp bias
TCOMP = 0.0            # fp32r truncation bias compensation on t (B0-row
                       # removal already eliminated the systematic bias)

f32 = np.float32

# phase-2 packed t/er/y layout: (chunk I, start, width); j range of chunk I
# rows is [I*128, 512) so widths are 512, 384, 128, 256 packed tightly.
SEG = [(0, 0, 512), (1, 512, 384), (3, 896, 128), (2, 1024, 256)]
WTOT = 1280

_CACHE = {}


def _build_program():
    import concourse.bacc as bacc
    import concourse.mybir as mybir
    import concourse.tile as tile

    nc = bacc.Bacc("TRN2", target_bir_lowering=False, debug=False,
                   num_devices=NCORES)
    dt = mybir.dt
    AF = mybir.ActivationFunctionType
    ALU = mybir.AluOpType

    xh_d = nc.declare_dram_parameter("xh", (K, BLOC * NP), dt.uint16, isOutput=False)
    bcol_d = nc.declare_dram_parameter("bcol", (K, 1), dt.float32, isOutput=False)
    wcol_d = nc.declare_dram_parameter("wcol", (128, BLOC * BLOC), dt.bfloat16, isOutput=False)
    w1t_d = nc.declare_dram_parameter("w1t", (NP, NUM_NODES), dt.float32, isOutput=False)
    b1p_d = nc.declare_dram_parameter("b1p", (1, NUM_NODES), dt.float32, isOutput=False)
    w2r_d = nc.declare_dram_parameter("w2r", (BLOC, NUM_NODES), dt.float32, isOutput=False)
    eye_d = nc.declare_dram_parameter("eye16", (16, 16), dt.float32, isOutput=False)
    y_d = nc.declare_dram_parameter("y", (BLOC, 1), dt.float32, isOutput=True)

    NCH = 4                       # xh DMA / Sin chunks
    CW = BLOC * NP // NCH         # 2048 cols per chunk

    with tile.TileContext(nc) as tc:
        with (
            tc.tile_pool(name="const", bufs=1) as cpool,
            tc.tile_pool(name="srowp", bufs=1, space="PSUM") as srowp,
        ):
            xh_s = cpool.tile([K, BLOC * NP], dt.uint16)
            for ch in range(NCH):
                nc.gpsimd.dma_start(xh_s[:, ch * CW:(ch + 1) * CW],
                                    xh_d[:, ch * CW:(ch + 1) * CW])
            bcol_s = cpool.tile([K, 1], dt.float32)
            nc.gpsimd.dma_start(bcol_s[:], bcol_d[:])
            wcol_s = cpool.tile([128, BLOC * BLOC], dt.bfloat16)
            nc.gpsimd.dma_start(wcol_s[:], wcol_d[:])
            w1t_s = cpool.tile([128, 4 * NUM_NODES], dt.float32r)
            for c in range(4):
                nc.gpsimd.dma_start(
                    w1t_s[:, c * NUM_NODES:(c + 1) * NUM_NODES],
                    w1t_d[c * 128:(c + 1) * 128, :])
            b1p_s = cpool.tile([1, NUM_NODES], dt.float32)
            nc.gpsimd.dma_start(b1p_s[:], b1p_d[:])
            w2r_s = cpool.tile([BLOC, NUM_NODES], dt.float32)
            nc.gpsimd.dma_start(w2r_s[:], w2r_d[:])
            eye_s = cpool.tile([16, 16], dt.float32)
            nc.gpsimd.dma_start(eye_s[:], eye_d[:])
            ones1_s = cpool.tile([1, BLOC], dt.float32)
            nc.gpsimd.memset(ones1_s[:], 1.0)
            sbias_s = cpool.tile([K, 1], dt.float32)
            nc.gpsimd.memset(sbias_s[:], -PI)
            ebias_s = cpool.tile([128, 1], dt.float32)
            nc.gpsimd.memset(ebias_s[:], -A * (B0C - TCOMP))

            E_s = cpool.tile([K, BLOC * NP], dt.float32r, name="E")
            Ew_s = cpool.tile([K, BLOC * NP], dt.float32r, name="Ew")
            acc = [cpool.tile([128, BLOC], dt.float32, name=f"acc{i}")
                   for i in range(4)]
            # column sums accumulate here: row b = batch b, js 128..512
            srow = srowp.tile([BLOC, NP], dt.float32)

            # ---------------- phase 1: embeddings ----------------
            for ch in range(NCH):
                nc.scalar.activation(E_s[:, ch * CW:(ch + 1) * CW],
                                     xh_s[:, ch * CW:(ch + 1) * CW],
                                     AF.Sin, scale=2.0 * PI / 65536.0,
                                     bias=sbias_s[:, 0:1])
                nc.vector.tensor_scalar(Ew_s[:, ch * CW:(ch + 1) * CW],
                                        E_s[:, ch * CW:(ch + 1) * CW],
                                        bcol_s[:, 0:1], None, ALU.mult)

            tc.no_sync_barrier()

            # ---------------- phase 2: pair blocks ----------------
            with (
                tc.tile_pool(name="tpsum", bufs=2, space="PSUM") as tpsum,
                tc.tile_pool(name="er", bufs=2) as erpool,
                tc.tile_pool(name="yy", bufs=2) as ypool,
            ):
                for b in range(BLOC):
                    o = b * NP
                    t = tpsum.tile([128, WTOT], dt.float32, tag="t")
                    for I, s, w in SEG:
                        nc.tensor.matmul(
                            t[:, s:s + w],
                            Ew_s[:, o + I * 128:o + (I + 1) * 128],
                            E_s[:, o + I * 128:o + NP],
                            start=True, stop=True, skip_group_check=True)
                    er = erpool.tile([128, WTOT], dt.bfloat16, tag="er")
                    nc.scalar.activation(er[:], t[:], AF.Exp, scale=-A,
                                         bias=ebias_s[:, 0:1])
                    y = ypool.tile([128, WTOT], dt.bfloat16, tag="y")
                    for I, s, w in SEG:
                        nc.vector.scalar_tensor_tensor(
                            y[:, s:s + w], er[:, s:s + w], CC, er[:, s:s + w],
                            ALU.add, ALU.mult, accum_out=acc[I][:, b:b + 1])
                    i_cs = 0
                    for I, s, w in SEG:
                        if w <= 128:
                            continue  # diagonal-only chunk: no column sums
                        # wsel_b: ones in column b -> result lands in row b,
                        # zeros accumulate harmlessly into the other rows
                        nc.tensor.matmul(
                            srow[:, (I + 1) * 128:NP],
                            wcol_s[:, b * BLOC:(b + 1) * BLOC],
                            y[:, s + 128:s + w],
                            start=(b == 0 and i_cs == 0),
                            stop=(b == BLOC - 1 and i_cs == 2),
                            skip_group_check=True)
                        i_cs += 1

            # ---------------- phase 3: recombine + MLP ----------------
            with (
                tc.tile_pool(name="trpsum", bufs=2, space="PSUM") as trpsum,
                tc.tile_pool(name="hpsum", bufs=1, space="PSUM") as hpsum,
                tc.tile_pool(name="tail", bufs=1) as tail,
            ):
                scopy = tail.tile([BLOC, 3 * 128], dt.float32)
                nc.vector.tensor_copy(scopy[:], srow[:, 128:NP])
                it = [tail.tile([128, BLOC], dt.float32r, name=f"it{i}")
                      for i in range(4)]
                nc.vector.tensor_copy(it[0][:], acc[0][:])
                for c in range(1, 4):
                    tp = trpsum.tile([128, BLOC], dt.float32, tag="tp")
                    nc.tensor.transpose(
                        tp[:], scopy[:, (c - 1) * 128:c * 128], eye_s[:])
                    nc.vector.tensor_tensor(it[c][:], tp[:], acc[c][:],
                                            ALU.add)
                h = hpsum.tile([BLOC, NUM_NODES], dt.float32)
                for c in range(4):
                    nc.tensor.matmul(
                        h[:], it[c][:],
                        w1t_s[:, c * NUM_NODES:(c + 1) * NUM_NODES],
                        start=(c == 0), stop=False, skip_group_check=True)
                nc.tensor.matmul(h[:], ones1_s[:], b1p_s[:],
                                 start=False, stop=True, skip_group_check=True)
                hr = tail.tile([BLOC, NUM_NODES], dt.float32)
                nc.scalar.activation(hr[:], h[:], AF.Relu)
                z = tail.tile([BLOC, 1], dt.float32)
                hw = tail.tile([BLOC, NUM_NODES], dt.float32)
                nc.vector.scalar_tensor_tensor(
                    hw[:], hr[:], 1.0, w2r_s[:], ALU.mult, ALU.mult,
                    accum_out=z[:])
                th = tail.tile([BLOC, 1], dt.float32)
                nc.scalar.activation(th[:], z[:], AF.Tanh, scale=0.5)
                ys = tail.tile([BLOC, 1], dt.float32)
                nc.vector.tensor_scalar(ys[:], th[:], 0.5, 0.5,
                                        ALU.mult, ALU.add)
                nc.gpsimd.dma_start(y_d[:], ys[:])

    nc.finalize()
    return nc


def _get_program():
    if "nc" not in _CACHE:
        _CACHE["nc"] = _build_program()
    return _CACHE["nc"]


def _host_xh(xs):
    """xs [BLOC, NP, 3] -> uint16 phase rows [K, BLOC*NP]."""
    nb = xs.shape[0]
    xh = np.empty((K, nb, NP), np.float64)
    xT = xs.astype(np.float64) / L          # [nb, NP, 3]
    col = 0
    for k in range(3):
        xk = xT[:, :, k]
        for n, psi, _w in _spec():
            base = n * xk + psi
            xh[col] = np.mod(base + 0.25, 1.0)      # cos row (phase .25)
            xh[col + 1] = np.mod(base, 1.0)          # sin row
            col += 2
    q = np.round((xh + 0.5) * 65536.0).astype(np.int64) % 65536
    return q.astype(np.uint16).reshape(K, nb * NP)


def _make_in_maps(x, W1, b1, W2):
    try:
        import ml_dtypes
        bf16 = ml_dtypes.bfloat16
    except ImportError:
        bf16 = None
    bcol = np.zeros((K, 1), f32)
    col = 0
    for k in range(3):
        for _n, _psi, w in _spec():
            bcol[col, 0] = w
            bcol[col + 1, 0] = w
            col += 2
    wcol = np.tile(np.eye(BLOC, dtype=f32).reshape(1, BLOC * BLOC),
                   (128, 1))
    wcol = np.ascontiguousarray(wcol)
    wcol = wcol.astype(bf16) if bf16 is not None else wcol
    W1 = np.asarray(W1, f32)
    w1t = np.ascontiguousarray((f32(EW2) * W1).T).astype(f32)
    # b1' = b1 + (NP*w0 - (w0 + w1 + w2)) * (W1 @ ones)
    corr = f32(NP * EW0 - (EW0 + EW1 + EW2))
    b1p = (np.asarray(b1, f32) + corr * W1.sum(axis=1)).reshape(1, NUM_NODES).astype(f32)
    w2r = np.broadcast_to(np.asarray(W2, f32).reshape(1, NUM_NODES),
                          (BLOC, NUM_NODES)).copy()
    eye16 = np.eye(16, dtype=f32)
    x = np.asarray(x, f32)
    in_maps = []
    for c in range(NCORES):
        xs = x[c * BLOC:(c + 1) * BLOC]
        in_maps.append({
            "xh": _host_xh(xs),
            "bcol": bcol, "wcol": wcol,
            "w1t": w1t, "b1p": b1p, "w2r": w2r, "eye16": eye16,
        })
    return in_maps


def kernel(x, W1, b1, W2, _trace=False, _trace_kwargs=None):
    from concourse.bass_utils import run_bass_kernel_spmd

    nc = _get_program()
    in_maps = _make_in_maps(x, W1, b1, W2)
    res = run_bass_kernel_spmd(nc, in_maps, list(range(NCORES)),
                               trace=_trace, **(_trace_kwargs or {}))
    out = np.concatenate([res.results[c]["y"] for c in range(NCORES)], axis=0)
    if _trace:
        _CACHE["last_result"] = res
    return out.astype(f32)
